# revision 12
# baseline (speedup 1.0000x reference)
"""Bass/Tile kernel for chunkwise retention (nn_ChunkwiseRetention).

Algorithm (per core = one batch element, seq 4000, B=5, 800 chunks):
superchunks of G=25 chunks (125 positions). The host pre-scales
xqT columns by g6^j and xkT by g6^-j (j = global chunk index), which
folds the entire cross-chunk decay into the projections: the cross
mask becomes 0/1, the carry is Q~ @ U with no rescale, and the state
update needs no scaling at all.

Everything on the PE runs in bf16 (1 cycle/row at any moving size in
the cost model, vs f32r's >=256-even constraint); PSUM accumulation
stays f32. Inputs are pre-cast to bf16 on the host (halves DMA bytes)
and packed into ONE dram tensor so each 4-superchunk group is a single
DMA (HWDGE is a serial ~625ns/DMA resource). Output walls are written
in pairs per DMA for the same reason.

Per superchunk s: Q~^T/K~^T projected per PAIR of superchunks (N=250)
into single-bank PSUM tiles (tag-rotated, bufs=2, so the ACT copy of
one pair overlaps the projection of the next); K~,V (pos-major)
projections; P~^T = K~ @ Q~^T (bf16, N=125); masked matmuls accumulate
cross + intra (+5-row shift via free-dim-shifted stationary) + seam
(previous superchunk's tail stationary x previous V) + carry (Q~ @ U)
into one PSUM window; running state U in one PSUM bank (zero-matmul
init, per-element has_written accumulation).

PSUM banks (8): qk 2x1 + kv 2 + pt 1 + wt 2 + u 1.
"""
import ml_dtypes
import numpy as np

import concourse.bass as bass
import concourse.mybir as mybir
import concourse.tile as tile

GAMMA = 0.9865
B = 5
SEQ = 4000
FEAT = 256
DIM = 256
G = 25
GP = G * B            # 125
NSC = SEQ // GP       # 32
NPAIR = NSC // 2      # 16
LG = 4                # superchunks per DMA load group
LGP = LG * GP         # 500
F32 = mybir.dt.float32
F32R = mybir.dt.float32r
BF16 = mybir.dt.bfloat16
NP_BF16 = ml_dtypes.bfloat16
g6 = float(np.float64(GAMMA) ** 6)
COPY = mybir.ActivationFunctionType.Copy

# const blob column layout
C_WCT = 0            # [0:125)   0/1 strict lower-block-triangular cross mask
C_WIT = 125          # [125:250) intra decay mask (rows 0:125)
C_Z = 250            # [250:762) zeros (row 0 used as zero matmul operand)
C_END = 762


def make_const_blob():
    t = np.arange(GP) // B
    p = np.arange(GP) % B
    tb, ta = t[:, None], t[None, :]
    wct01 = (tb < ta).astype(np.float32)
    qb, pa = p[:, None], p[None, :]
    wit = np.where((tb == ta) & (pa >= qb),
                   np.float64(GAMMA) ** (qb - pa), 0.0).astype(np.float32)
    blob = np.zeros((128, C_END), np.float32)
    blob[0:GP, C_WCT:C_WCT + 125] = wct01
    blob[0:GP, C_WIT:C_WIT + 125] = wit
    return blob


def build_kernel(nc: bass.Bass):
    xin = nc.dram_tensor("xin", [3, FEAT, SEQ], BF16, kind="ExternalInput").ap()
    wqkv = nc.dram_tensor("wqkv", [FEAT, 3 * DIM], BF16, kind="ExternalInput").ap()
    out = nc.dram_tensor("out", [SEQ, DIM], F32, kind="ExternalOutput").ap()

    blob_np = make_const_blob()
    mm = nc.tensor.matmul

    with tile.TileContext(nc) as tc:
        with (
            tc.tile_pool(name="consts", bufs=1) as cpool,
            tc.tile_pool(name="xin", bufs=2) as xpool,
            tc.tile_pool(name="work", bufs=2) as spool,
            tc.tile_pool(name="psQK", bufs=2, space="PSUM") as psQK,
            tc.tile_pool(name="psP", bufs=2, space="PSUM") as psP,
            tc.tile_pool(name="psPT", bufs=1, space="PSUM") as psPT,
            tc.tile_pool(name="psW", bufs=2, space="PSUM") as psW,
            tc.tile_pool(name="psU", bufs=1, space="PSUM") as psU,
        ):
            # --- constants to SBUF. Order matters at startup: weights
            # first (first projections need them), then the first
            # half-group of x, then the mask blob (first needed by the
            # DVE muls, ~2us in). ---
            w_sb = cpool.tile_from(wqkv.rearrange("(h p) d -> p h d", p=128))
            wk_sb = w_sb[:, :, 256:512]
            wv_sb = w_sb[:, :, 512:768]
            x0 = None  # filled by load_group(0) below
            blob_sb = cpool.tile([128, C_END], F32R, name="blob_sb")
            wct_sb = blob_sb[0:GP, C_WCT:C_WCT + 125]
            wit_sb = blob_sb[0:GP, C_WIT:C_WIT + 125]

            u_ps = psU.tile([128, 512], F32, name="u_state")

            # preamble: absorb the weights-DMA wait into one dummy matmul and
            # the const-blob DMA wait into one dummy DVE copy (fewer split
            # waits); zero-matmul initializes the U bank's data + has_written
            # bits so the per-superchunk state matmuls can all accumulate.
            nc.tensor.matmul(u_ps[0:1, 0:1], w_sb[:, 0, 0:1], w_sb[:, 0, 0:1],
                             start=True, stop=True, skip_group_check=True)
            scratch_sb = spool.tile([1, 1], F32, name="scratch", tag="scratch")
            nc.vector.tensor_copy(scratch_sb, blob_sb[0:1, 0:1])
            mm(u_ps, blob_sb[0:1, C_Z:C_Z + 128], blob_sb[0:1, C_Z:C_Z + 512],
               start=True, stop=True, skip_group_check=True)

            # persistent mpi stationaries (manual double-buffer): zero columns
            # are memset once; the per-superchunk mul only rewrites cols 5:130
            mpi_bufs = []
            for i_ in range(3):
                mb_ = spool.tile([125, 250], BF16, name=f"mpi_{i_}", tag=f"mpi_{i_}",
                                 bufs=1)
                nc.vector.memset(mb_[:, 0:5], 0.0)
                nc.vector.memset(mb_[:, 130:250], 0.0)
                mpi_bufs.append(mb_)

            prev_mpi = prev_v = None
            wtpt = {}
            ut_tile = {}
            xg_tiles = {}
            pair_sb = {}

            def load_group(g, split=False):
                t = xpool.tile([128, 3, 2, LGP], BF16, name=f"x_{g}", tag="x")
                if split:
                    # first pair's columns first so projections start early
                    nc.sync.dma_start(
                        out=t[:, :, :, 0:250],
                        in_=xin[:, :, 0:250].rearrange(
                            "t (h p) a -> p t h a", p=128))
                    nc.sync.dma_start(out=blob_sb,
                                      in_=nc.inline_tensor(blob_np, "cblob")
                                      .ap().bitcast(F32R))
                    nc.sync.dma_start(
                        out=t[:, :, :, 250:500],
                        in_=xin[:, :, 250:500].rearrange(
                            "t (h p) a -> p t h a", p=128))
                else:
                    nc.sync.dma_start(
                        out=t,
                        in_=xin[:, :, g * LGP:(g + 1) * LGP].rearrange(
                            "t (h p) a -> p t h a", p=128))
                xg_tiles[g] = t

            def proj_pair(p):
                """Q~^T and K~^T (dim-major) for superchunks 2p, 2p+1."""
                g, pl = divmod(p, 2)
                xg = xg_tiles[g]
                csl = slice(pl * 2 * GP, (pl * 2 + 2) * GP)   # 250 cols
                qt2 = spool.tile([128, 2, 250], BF16, name=f"qt_{p}", tag="qt")
                kt2 = spool.tile([128, 2, 250], BF16, name=f"kt_{p}", tag="kt")
                for tidx, wlo, dst in ((0, 0, qt2), (1, 256, kt2)):
                    ps = psQK.tile([128, 2, 256], F32, name=f"qk_{tidx}_{p}",
                                   tag="qkps")
                    for dh in (0, 1):
                        for h in (0, 1):
                            mm(ps[:, dh, 0:250],
                               w_sb[:, h, wlo + dh * 128:wlo + dh * 128 + 128],
                               xg[:, tidx, h, csl],
                               start=(h == 0), stop=(h == 1))
                    nc.scalar.activation(dst, ps[:, :, 0:250], COPY)
                pair_sb[p] = (qt2, kt2)
                if p >= 2:
                    pair_sb.pop(p - 2, None)

            def prep_pt(s):
                """P~^T + masked stationaries for superchunk s (emitted an
                iteration early, so the DVE muls overlap the previous
                window)."""
                qt2, kt2 = pair_sb[s // 2]
                m = s % 2
                qlo = qt2[:, 0, m * GP:(m + 1) * GP]
                qhi = qt2[:, 1, m * GP:(m + 1) * GP]
                klo = kt2[:, 0, m * GP:(m + 1) * GP]
                khi = kt2[:, 1, m * GP:(m + 1) * GP]

                # P~^T = K~ @ Q~^T (bf16: 1 cycle/row at N=125)
                pt_ps = psPT.tile([125, 125], F32, name=f"pt_{s}", tag="pt")
                mm(pt_ps, klo, qlo, start=True, stop=False)
                mm(pt_ps, khi, qhi, start=False, stop=True)

                mpc_sb = spool.tile([125, 125], BF16, name=f"mpc_{s}", tag="mpc",
                                    bufs=4)
                mpi_sb = mpi_bufs[s % 3]
                nc.vector.tensor_mul(mpc_sb, pt_ps, wct_sb)
                nc.vector.tensor_mul(mpi_sb[:, 5:130], pt_ps, wit_sb)
                return dict(mpc_sb=mpc_sb, mpi_sb=mpi_sb, qlo=qlo, qhi=qhi)

            def prep_kv(s):
                """K~/V pos-major for superchunk s + prefetches."""
                g = s // LG
                xg = xg_tiles[g]
                lsl = slice((s % LG) * GP, (s % LG + 1) * GP)
                kv = psP.tile([125, 512], F32, name=f"kv_{s}", tag="kv")
                for h in (0, 1):
                    mm(kv[:, 0:256], xg[:, 1, h, lsl], wk_sb[:, h, :],
                       start=(h == 0), stop=(h == 1))
                for h in (0, 1):
                    mm(kv[:, 256:512], xg[:, 2, h, lsl], wv_sb[:, h, :],
                       start=(h == 0), stop=(h == 1))
                kv_sb = spool.tile([125, 512], BF16, name=f"kv_sb_{s}",
                                   tag="kvsb", bufs=5)
                k_sb = kv_sb[:, 0:256]
                v_sb = kv_sb[:, 256:512]
                nc.vector.tensor_copy(k_sb, kv[:, 0:256])
                nc.scalar.activation(v_sb, kv[:, 256:512], COPY)

                # prefetches: next x group at group boundaries, next qk pair
                # one pair ahead (so PT never waits on a fresh ACT copy)
                if s % LG == 0 and s // LG + 1 < SEQ // LGP:
                    load_group(s // LG + 1)
                if s % 2 == 1 and s // 2 + 2 < NPAIR:
                    proj_pair(s // 2 + 2)
                return dict(k_sb=k_sb, v_sb=v_sb)

            # prologue
            load_group(0, split=True)
            proj_pair(0)
            proj_pair(1)
            wtpt["t"] = psW.tile([125, 512], F32, name="wt_pre", tag="wt")
            stp = prep_pt(0)
            stk = prep_kv(0)
            wall_pair = None

            for s in range(NSC):
                k_sb, v_sb = stk["k_sb"], stk["v_sb"]
                mpc_sb, mpi_sb = stp["mpc_sb"], stp["mpi_sb"]
                qlo, qhi = stp["qlo"], stp["qhi"]

                # allocate this window's PSUM bank (wt cols 0:256, next PT in
                # cols 256:381) and emit PT+masks for s+1 FIRST: the DVE muls
                # then overlap this window's matmuls
                wtp = psW.tile([125, 512], F32, name=f"wt_{s}", tag="wt")
                wtpt["t"] = wtp
                wt = wtp[:, 0:256]
                if s + 1 < NSC:
                    stp = prep_pt(s + 1)

                if s == NSC - 1:
                    # final output chunk 799 = intra tail of the last
                    # superchunk; emitted before the last window so the
                    # kernel tail isn't serialized behind it
                    wtf = wtp[:, 256:512]
                    mm(wtf, mpi_sb[:, 125:250], v_sb, start=True, stop=True)
                    wallf_sb = spool.tile([5, 256], F32, name="wallf",
                                          tag="wallf")
                    nc.vector.tensor_copy(wallf_sb, wtf[0:5, 0:256])
                    nc.sync.dma_start(out=out[SEQ - B:SEQ], in_=wallf_sb)

                # --- window accumulation (one closed group). The seam
                # (intra tail of chunk s*G-1) is added directly from the
                # previous superchunk's tail stationary and V: rows 5:125 of
                # that matmul multiply zero columns and accumulate zeros. ---
                mm(wt, mpc_sb, v_sb, start=True, stop=False)
                mm(wt, mpi_sb[:, 0:125], v_sb, start=False, stop=(s == 0))
                if s > 0:
                    ut_sb = ut_tile["t"]
                    mm(wt, prev_mpi[:, 125:250], prev_v, start=False, stop=False)
                    mm(wt, qlo, ut_sb[:, 0:256], start=False, stop=False)
                    mm(wt, qhi, ut_sb[:, 256:512], start=False, stop=True)

                # --- state update (accumulates; U bank bits set by zero-mm) ---
                mm(u_ps[:, 0:256], k_sb[:, 0:128], v_sb,
                   start=False, stop=True, skip_group_check=True)
                mm(u_ps[:, 256:512], k_sb[:, 128:256], v_sb,
                   start=False, stop=True, skip_group_check=True)
                # U snapshot for iteration s+1's carry, emitted immediately so
                # it heads the ACT queue (it's on the state->carry recurrence)
                if s + 1 < NSC:
                    utn = spool.tile([128, 512], BF16, name=f"ut_{s + 1}",
                                     tag="ut", bufs=3)
                    nc.scalar.activation(utn, u_ps, COPY)
                    ut_tile["t"] = utn

                # K~/V for s+1 + prefetches
                if s + 1 < NSC:
                    stk = prep_kv(s + 1)

                # --- output (paired DMAs: (1,2),(3,4),...,(29,30); 0 and 31
                # single). HWDGE is a serial per-DMA overhead, so fewer DMAs. ---
                if s == 0:
                    w0 = spool.tile([125, 2, 256], F32, name="wall_0",
                                    tag="wall", bufs=3)
                    nc.vector.tensor_copy(w0[:, 0, :], wt)
                    nc.sync.dma_start(out=out[0:GP - B], in_=w0[B:GP, 0, :])
                elif s == NSC - 1:
                    wl = spool.tile([125, 2, 256], F32, name=f"wall_{s}",
                                    tag="wall", bufs=3)
                    nc.vector.tensor_copy(wl[:, 0, :], wt)
                    nc.sync.dma_start(out=out[s * GP - B: s * GP - B + GP],
                                      in_=wl[:, 0, :])
                elif s % 2 == 1:
                    wall_pair = spool.tile([125, 2, 256], F32, name=f"wall_{s}",
                                           tag="wall", bufs=3)
                    nc.vector.tensor_copy(wall_pair[:, 0, :], wt)
                else:
                    nc.vector.tensor_copy(wall_pair[:, 1, :], wt)
                    base = (s - 1) * GP - B
                    nc.sync.dma_start(
                        out=out[base:base + 2 * GP].rearrange(
                            "(b p) d -> p b d", b=2),
                        in_=wall_pair)
                prev_mpi, prev_v = mpi_sb, v_sb

    return nc


def _col_scales():
    j = np.arange(SEQ) // B          # global chunk index
    sq = (np.float64(g6) ** j).astype(np.float32)
    sk = (np.float64(g6) ** (-j)).astype(np.float32)
    return sq, sk


def prep_core_inputs(xq2d, xk2d, xv2d, wqkv):
    sq, sk = _col_scales()
    xin = np.stack([
        (xq2d.T * sq[None, :]).astype(NP_BF16),
        (xk2d.T * sk[None, :]).astype(NP_BF16),
        xv2d.T.astype(NP_BF16),
    ], axis=0)
    return {
        "xin": np.ascontiguousarray(xin),
        "wqkv": wqkv.astype(NP_BF16),
    }


def make_in_maps(inputs):
    """inputs: dict from setup_inputs (full batch). Returns per-core in_maps."""
    xq, xk, xv = inputs["xq"], inputs["xk"], inputs["xv"]
    wqkv = np.ascontiguousarray(np.concatenate(
        [np.asarray(inputs["Wq"], dtype=np.float32),
         np.asarray(inputs["Wk"], dtype=np.float32),
         np.asarray(inputs["Wv"], dtype=np.float32)], axis=1))
    in_maps = []
    for b in range(8):
        in_maps.append(prep_core_inputs(
            np.asarray(xq[b], dtype=np.float32),
            np.asarray(xk[b], dtype=np.float32),
            np.asarray(xv[b], dtype=np.float32), wqkv))
    return in_maps


_NC_CACHE = {}


def _get_nc():
    if "nc" not in _NC_CACHE:
        from concourse import bacc
        nc = bacc.Bacc("TRN2", target_bir_lowering=False, debug=False)
        build_kernel(nc)
        nc.compile()
        _NC_CACHE["nc"] = nc
    return _NC_CACHE["nc"]


def run(inputs, trace=False, **kwargs):
    """Run on 8 NeuronCores; returns (output [8,4000,256], BassKernelResults)."""
    from concourse.bass_utils import run_bass_kernel_spmd

    nc = _get_nc()
    in_maps = make_in_maps(inputs)
    res = run_bass_kernel_spmd(nc, in_maps, core_ids=list(range(8)),
                               trace=trace, **kwargs)
    out = np.stack([r["out"] for r in res.results], axis=0)
    return out, res


def kernel(**inputs) -> np.ndarray:
    out, _ = run(inputs)
    return out


# revision 13
# speedup vs baseline: 1.0522x; 1.0522x over previous
"""Bass/Tile kernel for chunkwise retention (nn_ChunkwiseRetention).

Algorithm (per core = one batch element, seq 4000, B=5, 800 chunks):
superchunks of G=25 chunks (125 positions). The host pre-scales
xqT columns by g6^j and xkT by g6^-j (j = global chunk index), which
folds the entire cross-chunk decay into the projections: the cross
mask becomes 0/1, the carry is Q~ @ U with no rescale, and the state
update needs no scaling at all.

Everything on the PE runs in bf16 (1 cycle/row at any moving size in
the cost model, vs f32r's >=256-even constraint); PSUM accumulation
stays f32. Inputs are pre-cast to bf16 on the host (halves DMA bytes)
and packed into ONE dram tensor so each 4-superchunk group is a single
DMA (HWDGE is a serial ~625ns/DMA resource). Output walls are written
in pairs per DMA for the same reason.

Per superchunk s: Q~^T/K~^T projected per PAIR of superchunks (N=250)
into single-bank PSUM tiles (tag-rotated, bufs=2, so the ACT copy of
one pair overlaps the projection of the next); K~,V (pos-major)
projections; P~^T = K~ @ Q~^T (bf16, N=125); masked matmuls accumulate
cross + intra (+5-row shift via free-dim-shifted stationary) + seam
(previous superchunk's tail stationary x previous V) + carry (Q~ @ U)
into one PSUM window; running state U in one PSUM bank (zero-matmul
init, per-element has_written accumulation).

PSUM banks (8): qk 2x1 + kv 2 + pt 1 + wt 2 + u 1.
"""
import ml_dtypes
import numpy as np

import concourse.bass as bass
import concourse.mybir as mybir
import concourse.tile as tile

GAMMA = 0.9865
B = 5
SEQ = 4000
FEAT = 256
DIM = 256
G = 25
GP = G * B            # 125
NSC = SEQ // GP       # 32
NPAIR = NSC // 2      # 16
LG = 4                # superchunks per DMA load group
LGP = LG * GP         # 500
F32 = mybir.dt.float32
F32R = mybir.dt.float32r
BF16 = mybir.dt.bfloat16
NP_BF16 = ml_dtypes.bfloat16
g6 = float(np.float64(GAMMA) ** 6)
COPY = mybir.ActivationFunctionType.Copy

# const blob column layout
C_WCT = 0            # [0:125)   0/1 strict lower-block-triangular cross mask
C_WIT = 125          # [125:250) intra decay mask (rows 0:125)
C_Z = 250            # [250:762) zeros (row 0 used as zero matmul operand)
C_END = 762


def make_const_blob():
    t = np.arange(GP) // B
    p = np.arange(GP) % B
    tb, ta = t[:, None], t[None, :]
    wct01 = (tb < ta).astype(np.float32)
    qb, pa = p[:, None], p[None, :]
    wit = np.where((tb == ta) & (pa >= qb),
                   np.float64(GAMMA) ** (qb - pa), 0.0).astype(np.float32)
    blob = np.zeros((128, C_END), np.float32)
    blob[0:GP, C_WCT:C_WCT + 125] = wct01
    blob[0:GP, C_WIT:C_WIT + 125] = wit
    return blob


def build_kernel(nc: bass.Bass):
    xin = nc.dram_tensor("xin", [3, FEAT, SEQ], BF16, kind="ExternalInput").ap()
    wqkv = nc.dram_tensor("wqkv", [FEAT, 3 * DIM], BF16, kind="ExternalInput").ap()
    out = nc.dram_tensor("out", [SEQ, DIM], F32, kind="ExternalOutput").ap()

    blob_np = make_const_blob()
    mm = nc.tensor.matmul

    with tile.TileContext(nc) as tc:
        with (
            tc.tile_pool(name="consts", bufs=1) as cpool,
            tc.tile_pool(name="xin", bufs=2) as xpool,
            tc.tile_pool(name="work", bufs=2) as spool,
            tc.tile_pool(name="psQK", bufs=2, space="PSUM") as psQK,
            tc.tile_pool(name="psP", bufs=3, space="PSUM") as psP,
            tc.tile_pool(name="psW", bufs=2, space="PSUM") as psW,
            tc.tile_pool(name="psU", bufs=1, space="PSUM") as psU,
        ):
            # --- constants to SBUF. Order matters at startup: weights
            # first (first projections need them), then the first
            # half-group of x, then the mask blob (first needed by the
            # DVE muls, ~2us in). ---
            w_sb = cpool.tile_from(wqkv.rearrange("(h p) d -> p h d", p=128))
            wk_sb = w_sb[:, :, 256:512]
            wv_sb = w_sb[:, :, 512:768]
            x0 = None  # filled by load_group(0) below
            blob_sb = cpool.tile([128, C_END], F32R, name="blob_sb")
            wct_sb = blob_sb[0:GP, C_WCT:C_WCT + 125]
            wit_sb = blob_sb[0:GP, C_WIT:C_WIT + 125]

            u_ps = psU.tile([128, 512], F32, name="u_state")

            # preamble: absorb the weights-DMA wait into one dummy matmul and
            # the const-blob DMA wait into one dummy DVE copy (fewer split
            # waits); zero-matmul initializes the U bank's data + has_written
            # bits so the per-superchunk state matmuls can all accumulate.
            nc.tensor.matmul(u_ps[0:1, 0:1], w_sb[:, 0, 0:1], w_sb[:, 0, 0:1],
                             start=True, stop=True, skip_group_check=True)
            scratch_sb = spool.tile([1, 1], F32, name="scratch", tag="scratch")
            nc.vector.tensor_copy(scratch_sb, blob_sb[0:1, 0:1])
            mm(u_ps, blob_sb[0:1, C_Z:C_Z + 128], blob_sb[0:1, C_Z:C_Z + 512],
               start=True, stop=True, skip_group_check=True)

            # persistent mpi stationaries (manual double-buffer): zero columns
            # are memset once; the per-superchunk mul only rewrites cols 5:130
            mpi_bufs = []
            for i_ in range(3):
                mb_ = spool.tile([125, 250], BF16, name=f"mpi_{i_}", tag=f"mpi_{i_}",
                                 bufs=1)
                nc.vector.memset(mb_[:, 0:5], 0.0)
                nc.vector.memset(mb_[:, 130:250], 0.0)
                mpi_bufs.append(mb_)

            prev_mpi = prev_v = None
            wtpt = {}
            ut_tile = {}
            xg_tiles = {}
            pair_sb = {}

            def load_group(g, split=False):
                t = xpool.tile([128, 3, 2, LGP], BF16, name=f"x_{g}", tag="x")
                if split:
                    # first pair's columns first so projections start early
                    nc.sync.dma_start(
                        out=t[:, :, :, 0:250],
                        in_=xin[:, :, 0:250].rearrange(
                            "t (h p) a -> p t h a", p=128))
                    nc.sync.dma_start(out=blob_sb,
                                      in_=nc.inline_tensor(blob_np, "cblob")
                                      .ap().bitcast(F32R))
                    nc.sync.dma_start(
                        out=t[:, :, :, 250:500],
                        in_=xin[:, :, 250:500].rearrange(
                            "t (h p) a -> p t h a", p=128))
                else:
                    nc.sync.dma_start(
                        out=t,
                        in_=xin[:, :, g * LGP:(g + 1) * LGP].rearrange(
                            "t (h p) a -> p t h a", p=128))
                xg_tiles[g] = t

            def proj_pair(p):
                """Q~^T and K~^T (dim-major) for superchunks 2p, 2p+1."""
                g, pl = divmod(p, 2)
                xg = xg_tiles[g]
                csl = slice(pl * 2 * GP, (pl * 2 + 2) * GP)   # 250 cols
                qt2 = spool.tile([128, 2, 250], BF16, name=f"qt_{p}", tag="qt")
                kt2 = spool.tile([128, 2, 250], BF16, name=f"kt_{p}", tag="kt")
                for tidx, wlo, dst in ((0, 0, qt2), (1, 256, kt2)):
                    ps = psQK.tile([128, 2, 256], F32, name=f"qk_{tidx}_{p}",
                                   tag="qkps")
                    for dh in (0, 1):
                        for h in (0, 1):
                            mm(ps[:, dh, 0:250],
                               w_sb[:, h, wlo + dh * 128:wlo + dh * 128 + 128],
                               xg[:, tidx, h, csl],
                               start=(h == 0), stop=(h == 1))
                    nc.scalar.activation(dst, ps[:, :, 0:250], COPY)
                pair_sb[p] = (qt2, kt2)
                if p >= 2:
                    pair_sb.pop(p - 2, None)

            def prep_pt(s):
                """P~^T + masked stationaries for superchunk s (emitted an
                iteration early, so the DVE muls overlap the previous
                window)."""
                qt2, kt2 = pair_sb[s // 2]
                m = s % 2
                qlo = qt2[:, 0, m * GP:(m + 1) * GP]
                qhi = qt2[:, 1, m * GP:(m + 1) * GP]
                klo = kt2[:, 0, m * GP:(m + 1) * GP]
                khi = kt2[:, 1, m * GP:(m + 1) * GP]

                # P~^T = K~ @ Q~^T (bf16: 1 cycle/row at N=125) into the
                # spare half of THIS superchunk's window bank (allocated here,
                # one iteration ahead of the window matmuls)
                wtp = psW.tile([125, 512], F32, name=f"wt_{s}", tag="wt")
                wtpt[s] = wtp
                pt_ps = wtp[:, 256:381]
                mm(pt_ps, klo, qlo, start=True, stop=False)
                mm(pt_ps, khi, qhi, start=False, stop=True)

                mpc_sb = spool.tile([125, 125], BF16, name=f"mpc_{s}", tag="mpc",
                                    bufs=4)
                mpi_sb = mpi_bufs[s % 3]
                nc.vector.tensor_mul(mpc_sb, pt_ps, wct_sb)
                nc.vector.tensor_mul(mpi_sb[:, 5:130], pt_ps, wit_sb)
                return dict(mpc_sb=mpc_sb, mpi_sb=mpi_sb, qlo=qlo, qhi=qhi)

            def prep_kv(s):
                """K~/V pos-major for superchunk s + prefetches."""
                g = s // LG
                xg = xg_tiles[g]
                lsl = slice((s % LG) * GP, (s % LG + 1) * GP)
                kv = psP.tile([125, 512], F32, name=f"kv_{s}", tag="kv")
                for h in (0, 1):
                    mm(kv[:, 0:256], xg[:, 1, h, lsl], wk_sb[:, h, :],
                       start=(h == 0), stop=(h == 1))
                for h in (0, 1):
                    mm(kv[:, 256:512], xg[:, 2, h, lsl], wv_sb[:, h, :],
                       start=(h == 0), stop=(h == 1))
                kv_sb = spool.tile([125, 512], BF16, name=f"kv_sb_{s}",
                                   tag="kvsb", bufs=5)
                k_sb = kv_sb[:, 0:256]
                v_sb = kv_sb[:, 256:512]
                nc.vector.tensor_copy(k_sb, kv[:, 0:256])
                nc.scalar.activation(v_sb, kv[:, 256:512], COPY)

                # prefetches: next x group at group boundaries, next qk pair
                # one pair ahead (so PT never waits on a fresh ACT copy)
                if s % LG == 0 and s // LG + 1 < SEQ // LGP:
                    load_group(s // LG + 1)
                if s % 2 == 1 and s // 2 + 2 < NPAIR:
                    proj_pair(s // 2 + 2)
                return dict(k_sb=k_sb, v_sb=v_sb)

            # prologue
            load_group(0, split=True)
            proj_pair(0)
            proj_pair(1)
            stp = prep_pt(0)
            stk = prep_kv(0)
            wall_pair = None

            for s in range(NSC):
                k_sb, v_sb = stk["k_sb"], stk["v_sb"]
                mpc_sb, mpi_sb = stp["mpc_sb"], stp["mpi_sb"]
                qlo, qhi = stp["qlo"], stp["qhi"]

                # this window's PSUM bank was allocated by prep_pt(s) an
                # iteration ago (wt cols 0:256, its PT came in cols 256:381).
                # Emit PT+masks for s+1 FIRST: the DVE muls then overlap this
                # window's matmuls.
                wtp = wtpt.pop(s)
                wt = wtp[:, 0:256]
                if s + 1 < NSC:
                    stp = prep_pt(s + 1)

                if s == NSC - 1:
                    # final output chunk 799 = intra tail of the last
                    # superchunk; emitted before the last window so the
                    # kernel tail isn't serialized behind it
                    wtf = wtp[:, 256:512]
                    mm(wtf, mpi_sb[:, 125:250], v_sb, start=True, stop=True)
                    wallf_sb = spool.tile([5, 256], F32, name="wallf",
                                          tag="wallf")
                    nc.vector.tensor_copy(wallf_sb, wtf[0:5, 0:256])
                    nc.sync.dma_start(out=out[SEQ - B:SEQ], in_=wallf_sb)

                # --- window accumulation (one closed group). The seam
                # (intra tail of chunk s*G-1) is added directly from the
                # previous superchunk's tail stationary and V: rows 5:125 of
                # that matmul multiply zero columns and accumulate zeros. ---
                mm(wt, mpc_sb, v_sb, start=True, stop=False)
                mm(wt, mpi_sb[:, 0:125], v_sb, start=False, stop=(s == 0))
                if s > 0:
                    ut_sb = ut_tile["t"]
                    mm(wt, prev_mpi[:, 125:250], prev_v, start=False, stop=False)
                    mm(wt, qlo, ut_sb[:, 0:256], start=False, stop=False)
                    mm(wt, qhi, ut_sb[:, 256:512], start=False, stop=True)

                # --- state update (accumulates; U bank bits set by zero-mm) ---
                mm(u_ps[:, 0:256], k_sb[:, 0:128], v_sb,
                   start=False, stop=True, skip_group_check=True)
                mm(u_ps[:, 256:512], k_sb[:, 128:256], v_sb,
                   start=False, stop=True, skip_group_check=True)
                # U snapshot for iteration s+1's carry, emitted immediately so
                # it heads the ACT queue (it's on the state->carry recurrence)
                if s + 1 < NSC:
                    utn = spool.tile([128, 512], BF16, name=f"ut_{s + 1}",
                                     tag="ut", bufs=3)
                    nc.scalar.activation(utn, u_ps, COPY)
                    ut_tile["t"] = utn

                # K~/V for s+1 + prefetches
                if s + 1 < NSC:
                    stk = prep_kv(s + 1)

                # --- output (paired DMAs: (1,2),(3,4),...,(29,30); 0 and 31
                # single). HWDGE is a serial per-DMA overhead, so fewer DMAs. ---
                if s == 0:
                    w0 = spool.tile([125, 2, 256], F32, name="wall_0",
                                    tag="wall", bufs=3)
                    nc.vector.tensor_copy(w0[:, 0, :], wt)
                    nc.sync.dma_start(out=out[0:GP - B], in_=w0[B:GP, 0, :])
                elif s == NSC - 1:
                    wl = spool.tile([125, 2, 256], F32, name=f"wall_{s}",
                                    tag="wall", bufs=3)
                    nc.vector.tensor_copy(wl[:, 0, :], wt)
                    nc.sync.dma_start(out=out[s * GP - B: s * GP - B + GP],
                                      in_=wl[:, 0, :])
                elif s % 2 == 1:
                    wall_pair = spool.tile([125, 2, 256], F32, name=f"wall_{s}",
                                           tag="wall", bufs=3)
                    nc.vector.tensor_copy(wall_pair[:, 0, :], wt)
                else:
                    nc.vector.tensor_copy(wall_pair[:, 1, :], wt)
                    base = (s - 1) * GP - B
                    nc.sync.dma_start(
                        out=out[base:base + 2 * GP].rearrange(
                            "(b p) d -> p b d", b=2),
                        in_=wall_pair)
                prev_mpi, prev_v = mpi_sb, v_sb

    return nc


def _col_scales():
    j = np.arange(SEQ) // B          # global chunk index
    sq = (np.float64(g6) ** j).astype(np.float32)
    sk = (np.float64(g6) ** (-j)).astype(np.float32)
    return sq, sk


def prep_core_inputs(xq2d, xk2d, xv2d, wqkv):
    sq, sk = _col_scales()
    xin = np.stack([
        (xq2d.T * sq[None, :]).astype(NP_BF16),
        (xk2d.T * sk[None, :]).astype(NP_BF16),
        xv2d.T.astype(NP_BF16),
    ], axis=0)
    return {
        "xin": np.ascontiguousarray(xin),
        "wqkv": wqkv.astype(NP_BF16),
    }


def make_in_maps(inputs):
    """inputs: dict from setup_inputs (full batch). Returns per-core in_maps."""
    xq, xk, xv = inputs["xq"], inputs["xk"], inputs["xv"]
    wqkv = np.ascontiguousarray(np.concatenate(
        [np.asarray(inputs["Wq"], dtype=np.float32),
         np.asarray(inputs["Wk"], dtype=np.float32),
         np.asarray(inputs["Wv"], dtype=np.float32)], axis=1))
    in_maps = []
    for b in range(8):
        in_maps.append(prep_core_inputs(
            np.asarray(xq[b], dtype=np.float32),
            np.asarray(xk[b], dtype=np.float32),
            np.asarray(xv[b], dtype=np.float32), wqkv))
    return in_maps


_NC_CACHE = {}


def _get_nc():
    if "nc" not in _NC_CACHE:
        from concourse import bacc
        nc = bacc.Bacc("TRN2", target_bir_lowering=False, debug=False)
        build_kernel(nc)
        nc.compile()
        _NC_CACHE["nc"] = nc
    return _NC_CACHE["nc"]


def run(inputs, trace=False, **kwargs):
    """Run on 8 NeuronCores; returns (output [8,4000,256], BassKernelResults)."""
    from concourse.bass_utils import run_bass_kernel_spmd

    nc = _get_nc()
    in_maps = make_in_maps(inputs)
    res = run_bass_kernel_spmd(nc, in_maps, core_ids=list(range(8)),
                               trace=trace, **kwargs)
    out = np.stack([r["out"] for r in res.results], axis=0)
    return out, res


def kernel(**inputs) -> np.ndarray:
    out, _ = run(inputs)
    return out
